# revision 1
# baseline (speedup 1.0000x reference)
"""DeepSeekMoE Trainium2 kernel (8 NeuronCores, data-parallel over tokens).

Strategy
--------
Token-parallel: each of the 8 cores processes T/8 = 512 tokens end-to-end
(router + shared expert + all 8 experts dense + top-2 combine), so there are
no collectives; the host shards x and concatenates the 8 output shards.

Per-core compute layout (tokens t=512, D=1024, H=2048, E=8):
  - x [512,1024] is PE-transposed once into xT [1024,512] (fp32 copy for the
    router, fp32r copy for the expert matmuls).
  - Router logits run in full fp32 (top-2 selection is precision critical);
    the top-2 renormalized weights are sigmoid(+/-(l1-l2)) of the top-2
    logit gap, built with DVE max/is_equal masks (no exp, no reciprocal).
  - mm1:  hT[j] = gelu(ew1[e].T-block @ xT) accumulated in PSUM over the
    8 k-tiles, evicted via ScalarE Gelu (exact erf form) with cast to fp32r.
  - mm2:  out2 = hT-block.T @ ew2[e], accumulated in PSUM over 16 k-tiles,
    then fused into acc with one DVE op: acc += psum * comb[:,e] (per-token
    scalar). Shared expert initializes acc.
  - All big matmuls use float32r (full PE rate, ~12-bit mantissa); weights
    are pre-rounded to the fp32r grid on the host and declared float32r in
    DRAM so they stream over plain HWDGE DMAs.
  - Biases enter as K=1 / K=8 seed matmuls into the PSUM accumulation
    groups (ones (x) b row products); they are skipped entirely when the
    bias tensors are all-zero (the benchmark case).
"""

import os
import sys

sys.path.insert(0, "/opt/trn_rl_repo")

from contextlib import ExitStack

import numpy as np

import concourse.bass as bass  # noqa: F401  (engine types resolve through bacc)
import concourse.tile as tile
from concourse import bacc, mybir
from concourse.alu_op_type import AluOpType
from concourse.bass_utils import run_bass_kernel_spmd
from concourse.masks import make_identity

F32 = mybir.dt.float32
F32R = mybir.dt.float32r
AF = mybir.ActivationFunctionType

D, H, E = 1024, 2048, 8
B, S = 2, 2048
T = B * S
NCORES = 8
TC = T // NCORES          # 512 tokens per core
MT = TC // 128            # 4 token m-tiles
KD = D // 128             # 8 k-tiles over D
KH = H // 128             # 16 k-tiles over H
NQ = 4                    # hid quarters for mm1 psum
X = mybir.AxisListType.X


def _round_fp32r(a: np.ndarray) -> np.ndarray:
    """RNE-round fp32 values to the fp32r grid (low 11 mantissa bits zero)."""
    a = np.ascontiguousarray(a, dtype=np.float32)
    u = a.view(np.uint32)
    r = (u + 0x3FF + ((u >> 11) & 1)) & np.uint32(0xFFFFF800)
    return r.astype(np.uint32).view(np.float32).reshape(a.shape)


def build_program(has_b1: bool, has_b2: bool, has_rb: bool):
    nc = bacc.Bacc("TRN2", debug=False)

    x = nc.dram_tensor("x", [TC, D], F32, kind="ExternalInput").ap()
    rw = nc.dram_tensor("router_w", [D, E], F32, kind="ExternalInput").ap()
    rb = nc.dram_tensor("router_b", [1, E], F32, kind="ExternalInput").ap()
    sw1 = nc.dram_tensor("sw1", [D, H], F32R, kind="ExternalInput").ap()
    sb1 = nc.dram_tensor("sb1", [1, H], F32R, kind="ExternalInput").ap()
    sw2 = nc.dram_tensor("sw2", [H, D], F32R, kind="ExternalInput").ap()
    sb2 = nc.dram_tensor("sb2", [1, D], F32R, kind="ExternalInput").ap()
    ew1 = nc.dram_tensor("ew1", [E, D, H], F32R, kind="ExternalInput").ap()
    eb1 = nc.dram_tensor("eb1", [E, H], F32R, kind="ExternalInput").ap()
    ew2 = nc.dram_tensor("ew2", [E, H, D], F32R, kind="ExternalInput").ap()
    eb2 = nc.dram_tensor("eb2", [E, D], F32R, kind="ExternalInput").ap()
    out = nc.dram_tensor("out", [TC, D], F32, kind="ExternalOutput").ap()

    with tile.TileContext(nc) as tc, ExitStack() as ctx:
        const = ctx.enter_context(tc.tile_pool(name="const", bufs=1))
        xpool = ctx.enter_context(tc.tile_pool(name="xpool", bufs=1))
        rpool = ctx.enter_context(tc.tile_pool(name="rpool", bufs=2))
        any_bias = has_b1 or has_b2
        w1p = ctx.enter_context(tc.tile_pool(name="w1p", bufs=8 if any_bias else 10))
        w2p = ctx.enter_context(tc.tile_pool(name="w2p", bufs=4 if any_bias else 6))
        htp = ctx.enter_context(tc.tile_pool(name="htp", bufs=1 if any_bias else 2))
        if has_b1:
            b1p = ctx.enter_context(tc.tile_pool(name="b1p", bufs=2))
        accp = ctx.enter_context(tc.tile_pool(name="accp", bufs=1))
        psp = ctx.enter_context(tc.tile_pool(name="psp", bufs=8, space="PSUM"))

        # ---- constants ----
        nonce = float(os.environ.get("KERNEL_BUILD_NONCE", "0") or 0)
        if nonce:
            scratch = const.tile([128, 1], F32, tag="nonce")
            nc.vector.memset(scratch, nonce)
        ident = const.tile([128, 128], F32, tag="ident")
        make_identity(nc, ident)
        rw_sb = const.tile([128, KD, E], F32, tag="rw")
        nc.sync.dma_start(out=rw_sb, in_=rw.rearrange("(k p) e -> p k e", p=128))

        ones_f = const.tile([1, 128], F32, tag="ones_f")
        nc.vector.memset(ones_f, 1.0)
        if has_rb:
            rb_sb = const.tile([1, E], F32, tag="rb")
            nc.sync.dma_start(out=rb_sb, in_=rb)
        if has_b1:
            ones_r = const.tile([1, TC], F32R, tag="ones_r")
            ones_ftc = const.tile([1, TC], F32, tag="ones_ftc")
            nc.vector.memset(ones_ftc, 1.0)
            nc.vector.tensor_copy(ones_r, ones_ftc[:])
        if has_b2:
            onesm_r = const.tile([1, 128], F32R, tag="onesm_r")
            nc.vector.tensor_copy(onesm_r, ones_f[:])
            sb2_sb = const.tile([1, D], F32R, tag="sb2")
            nc.sync.dma_start(out=sb2_sb, in_=sb2)
            eb2_sb = const.tile([E, D], F32R, tag="eb2")
            nc.sync.dma_start(out=eb2_sb, in_=eb2)
            combT = const.tile([32, TC], F32R, tag="combT")

        acc = accp.tile([128, MT, D], F32, tag="acc")

        # ---- load x, transpose to xT (fp32 for router, fp32r for mm1) ----
        x_sb = []
        for m in range(MT):
            xt = xpool.tile([128, D], F32, tag=f"x{m}", name=f"x_sb{m}")
            nc.sync.dma_start(out=xt, in_=x[m * 128 : (m + 1) * 128, :])
            x_sb.append(xt)
        xT_r = [xpool.tile([128, TC], F32R, tag=f"xtr{k}", name=f"xT_r{k}") for k in range(KD)]
        xT_f = [xpool.tile([128, TC], F32, tag=f"xtf{k}", name=f"xT_f{k}") for k in range(KD)]
        for m in range(MT):
            for k in range(KD):
                pt = psp.tile([128, 128], F32, tag="ps", name=f"pt{m}_{k}")
                nc.tensor.transpose(pt, x_sb[m][:, k * 128 : (k + 1) * 128], ident[:])
                nc.vector.tensor_copy(xT_r[k][:, m * 128 : (m + 1) * 128], pt[:])
                nc.scalar.copy(xT_f[k][:, m * 128 : (m + 1) * 128], pt[:])

        # ---- router: logits (full fp32) -> top-2 sigmoid combine weights ----
        comb = []
        for m in range(MT):
            lp = psp.tile([128, E], F32, tag="ps", name=f"lp{m}")
            for k in range(KD):
                nc.tensor.matmul(
                    lp,
                    xT_f[k][:, m * 128 : (m + 1) * 128],
                    rw_sb[:, k, :],
                    start=(k == 0),
                    stop=(k == KD - 1 and not has_rb),
                )
            if has_rb:
                nc.tensor.matmul(lp, ones_f[:], rb_sb[:], start=False, stop=True)

            l_sb = rpool.tile([128, E], F32, tag="l", name=f"l{m}")
            nc.vector.tensor_copy(l_sb, lp[:])
            m1 = rpool.tile([128, 1], F32, tag="m1", name=f"m1_{m}")
            nc.vector.reduce_max(m1, l_sb[:], axis=X)
            mask1 = rpool.tile([128, E], F32, tag="mask1", name=f"mask1_{m}")
            nc.vector.tensor_scalar(mask1, l_sb[:], m1[:], None, op0=AluOpType.is_equal)
            lm = rpool.tile([128, E], F32, tag="lm", name=f"lm{m}")
            nc.vector.scalar_tensor_tensor(
                out=lm, in0=mask1[:], scalar=-1e30, in1=l_sb[:],
                op0=AluOpType.mult, op1=AluOpType.add)
            m2 = rpool.tile([128, 1], F32, tag="m2", name=f"m2_{m}")
            nc.vector.reduce_max(m2, lm[:], axis=X)
            mask2 = rpool.tile([128, E], F32, tag="mask2", name=f"mask2_{m}")
            nc.vector.tensor_scalar(mask2, lm[:], m2[:], None, op0=AluOpType.is_equal)
            dgap = rpool.tile([128, 1], F32, tag="dgap", name=f"dgap{m}")
            nc.vector.tensor_tensor(dgap, m1[:], m2[:], op=AluOpType.subtract)
            s1 = rpool.tile([128, 1], F32, tag="s1", name=f"s1_{m}")
            nc.scalar.activation(s1, dgap[:], AF.Sigmoid)
            s2 = rpool.tile([128, 1], F32, tag="s2", name=f"s2_{m}")
            nc.scalar.activation(s2, dgap[:], AF.Sigmoid, scale=-1.0)
            c1 = rpool.tile([128, E], F32, tag="c1", name=f"c1_{m}")
            nc.vector.tensor_scalar(c1, mask1[:], s1[:], None, op0=AluOpType.mult)
            cm = const.tile([128, E], F32, tag=f"comb{m}", name=f"comb{m}")
            nc.vector.scalar_tensor_tensor(
                out=cm, in0=mask2[:], scalar=s2[:], in1=c1[:],
                op0=AluOpType.mult, op1=AluOpType.add)
            comb.append(cm)

            if has_b2:
                c32 = rpool.tile([128, 32], F32, tag="c32", name=f"c32_{m}")
                nc.vector.memset(c32, 0.0)
                nc.vector.tensor_copy(c32[:, 0:E], cm[:])
                pct = psp.tile([32, 128], F32, tag="ps", name=f"pct{m}")
                nc.tensor.transpose(pct, c32[:], ident[:])
                nc.vector.tensor_copy(combT[:, m * 128 : (m + 1) * 128], pct[:])

        # ---- shared expert + 8 routed experts ----
        for mat in range(E + 1):
            is_shared = mat == 0
            e = mat - 1
            w1ap = sw1 if is_shared else ew1[e]
            w2ap = sw2 if is_shared else ew2[e]
            if has_b1:
                b1row = b1p.tile([1, H], F32R, tag="b1", name=f"b1_{mat}")
                nc.sync.dma_start(
                    out=b1row, in_=(sb1 if is_shared else eb1[e : e + 1, :]))

            # mm1: hT[j] = gelu(w1.T @ xT) in hid quarters of 4 psum banks.
            # w1 streams as 1MB quad-k DMAs: [128, 4, 512] covers k=4g..4g+3.
            hts = []
            for q in range(NQ):
                phs = []
                for mh in range(4):
                    ph = psp.tile([128, TC], F32, tag="ps", name=f"ph{mat}_{q}_{mh}")
                    phs.append(ph)
                    if has_b1:
                        j = q * 4 + mh
                        nc.tensor.matmul(
                            ph, b1row[:, j * 128 : (j + 1) * 128], ones_r[:],
                            start=True, stop=False)
                for k in range(KD):
                    w1t = w1p.tile([128, 512], F32R, tag="w1", name=f"w1_{mat}_{q}_{k}")
                    nc.sync.dma_start(
                        out=w1t,
                        in_=w1ap[k * 128 : (k + 1) * 128, q * 512 : (q + 1) * 512])
                    for mh in range(4):
                        nc.tensor.matmul(
                            phs[mh],
                            w1t[:, mh * 128 : (mh + 1) * 128],
                            xT_r[k][:],
                            start=(k == 0 and not has_b1),
                            stop=(k == KD - 1))
                for mh in range(4):
                    j = q * 4 + mh
                    ht = htp.tile([128, TC], F32R, tag=f"ht{j}", name=f"ht{mat}_{j}")
                    nc.scalar.activation(ht, phs[mh][:], AF.Gelu)
                    hts.append(ht)

            # mm2: psum[mt,n] = sum_k hT[k][:,mt].T @ w2[k][:,n]
            seeded = is_shared and has_b2
            pos = []
            for mt in range(MT):
                for n in range(2):
                    po = psp.tile([128, 512], F32, tag="ps", name=f"po{mat}_{mt}_{n}")
                    pos.append(po)
                    if seeded:
                        nc.tensor.matmul(
                            po, onesm_r[:], sb2_sb[:, n * 512 : (n + 1) * 512],
                            start=True, stop=False)
                        nc.tensor.matmul(
                            po, combT[0:E, mt * 128 : (mt + 1) * 128],
                            eb2_sb[:, n * 512 : (n + 1) * 512],
                            start=False, stop=False)
            for k in range(KH):
                w2t = w2p.tile([128, D], F32R, tag="w2", name=f"w2_{mat}_{k}")
                nc.sync.dma_start(out=w2t, in_=w2ap[k * 128 : (k + 1) * 128, :])
                for mt in range(MT):
                    for n in range(2):
                        nc.tensor.matmul(
                            pos[mt * 2 + n],
                            hts[k][:, mt * 128 : (mt + 1) * 128],
                            w2t[:, n * 512 : (n + 1) * 512],
                            start=(k == 0 and not seeded),
                            stop=(k == KH - 1))

            # combine into acc
            for mt in range(MT):
                for n in range(2):
                    po = pos[mt * 2 + n]
                    dst = acc[:, mt, n * 512 : (n + 1) * 512]
                    if is_shared:
                        nc.vector.tensor_copy(dst, po[:])
                    else:
                        nc.vector.scalar_tensor_tensor(
                            out=dst, in0=po[:], scalar=comb[mt][:, e : e + 1],
                            in1=dst, op0=AluOpType.mult, op1=AluOpType.add)
                    if mat == E:
                        # last expert: stream each finished slice out so the
                        # store overlaps the remaining evicts instead of one
                        # 2MB DMA after the full chain.
                        nc.sync.dma_start(
                            out=out.rearrange("(m p) d -> p m d", p=128)[
                                :, mt, n * 512 : (n + 1) * 512],
                            in_=dst)

    nc.compile()
    return nc


_programs: dict = {}
LAST_RESULTS = None


def _get_program(key):
    if key not in _programs:
        _programs[key] = build_program(*key)
    return _programs[key]


def kernel(x, router_w, router_b, sw1, sb1, sw2, sb2, ew1, eb1, ew2, eb2):
    x = np.asarray(x, dtype=np.float32)
    flat = np.ascontiguousarray(x.reshape(T, D))
    has_b1 = bool(np.any(sb1)) or bool(np.any(eb1))
    has_b2 = bool(np.any(sb2)) or bool(np.any(eb2))
    has_rb = bool(np.any(router_b))

    nc = _get_program((has_b1, has_b2, has_rb))

    base = {
        "router_w": np.ascontiguousarray(np.asarray(router_w, np.float32)),
        "router_b": np.asarray(router_b, np.float32).reshape(1, E),
        "sw1": _round_fp32r(sw1),
        "sb1": _round_fp32r(np.asarray(sb1).reshape(1, H)),
        "sw2": _round_fp32r(sw2),
        "sb2": _round_fp32r(np.asarray(sb2).reshape(1, D)),
        "ew1": _round_fp32r(ew1),
        "eb1": _round_fp32r(eb1),
        "ew2": _round_fp32r(ew2),
        "eb2": _round_fp32r(eb2),
    }
    in_maps = [dict(base, x=flat[i * TC : (i + 1) * TC]) for i in range(NCORES)]
    res = None
    for attempt in range(3):
        try:
            res = run_bass_kernel_spmd(nc, in_maps, core_ids=list(range(NCORES)))
            break
        except Exception:
            if attempt == 2:
                raise
            import time as _time
            _time.sleep(5)  # transient device errors recover on retry
    global LAST_RESULTS
    LAST_RESULTS = res
    outs = [res.results[i]["out"] for i in range(NCORES)]
    return np.concatenate(outs, axis=0).reshape(B, S, D)

